# revision 7
# baseline (speedup 1.0000x reference)
"""NetVLAD pooling kernel for Trainium2 (Bass/Tile), 8-way data parallel over N.

Reference computation (per sample n):
    xf      = x[n].reshape(C, L)                      # C=512, L=784
    logits  = conv_w @ xf                             # (K=64, L)
    a       = softmax(logits, axis=0)                 # over clusters K
    agg     = a @ xf.T                                # (K, C)
    vlad    = agg - a.sum(-1)[:, None] * centroids    # (K, C)
    vlad   /= ||vlad||_2 rows                         # intra-norm over C
    v       = vlad.flatten() / ||vlad.flatten()||_2   # global norm

Layout strategy per core (8 samples):
  - x[s] loaded naturally as (128p, 4cc, 784l).
  - logits via PE with conv_w^T chunks stationary -> psum (64, 784), copy to SBUF.
  - logits transposed chunkwise on PE -> logitsT (l-part, K-free); softmax is then
    a free-dim reduction (max/exp/sum) per row.
  - x transposed chunkwise on PE (fp32 transpose mode, exact) -> xfT (l-part, C-free)
    with a ones column appended so the agg matmul also yields a.sum().
  - agg = a @ xfT via PE contracting l in 7 chunks of <=128.
  - epilogue: vlad, intra-norm (rsqrt via exp(-0.5*ln) to stay in one ACT table set),
    global norm folded in as rsqrt(64*ss) = rsqrt(ss)/8 (the global norm of the
    row-normalized vlad is 8 up to fp32 rounding).
"""

import numpy as np
from contextlib import ExitStack

import concourse.bass as bass
import concourse.tile as tile
from concourse import bacc, mybir
from concourse.bass_utils import run_bass_kernel_spmd
from concourse.masks import make_identity

N, C, HH, WW, K = 64, 512, 28, 28, 64
L = HH * WW            # 784
NCORES = 8
NS = N // NCORES       # 8 samples per core
CCN = C // 128         # 4 channel chunks
NLC = (L + 127) // 128  # 7 l-chunks (6x128 + 1x16)
F32 = mybir.dt.float32
F32R = mybir.dt.float32r
PS = bass.MemorySpace.PSUM


def build_program(n_iters: int = 1):
    nc = bacc.Bacc("TRN2", target_bir_lowering=False, debug=False)
    x_d = nc.dram_tensor("x", [NS, C, L], F32R, kind="ExternalInput")
    wt_d = nc.dram_tensor("conv_wt", [C, K], F32R, kind="ExternalInput")
    cent_d = nc.dram_tensor("cent", [K, C], F32, kind="ExternalInput")
    out_d = nc.dram_tensor("out", [NS, K * C], F32, kind="ExternalOutput")

    with tile.TileContext(nc) as tc, ExitStack() as ctx:
        _build_body(ctx, tc, x_d.ap(), wt_d.ap(), cent_d.ap(), out_d.ap(), n_iters)
    nc.compile()
    return nc


def _build_body(ctx, tc, x, wt, cent, out, n_iters):
    nc = tc.nc

    consts = ctx.enter_context(tc.tile_pool(name="consts", bufs=1))
    xf_pool = ctx.enter_context(tc.tile_pool(name="xf", bufs=2))
    xfT_pool = ctx.enter_context(tc.tile_pool(name="xfT", bufs=2))
    lg_pool = ctx.enter_context(tc.tile_pool(name="lg", bufs=2))
    aT_pool = ctx.enter_context(tc.tile_pool(name="aT", bufs=2))
    small = ctx.enter_context(tc.tile_pool(name="small", bufs=2))
    vlad_pool = ctx.enter_context(tc.tile_pool(name="vlad", bufs=2))
    out_pool = ctx.enter_context(tc.tile_pool(name="outp", bufs=2))
    scratch = ctx.enter_context(tc.tile_pool(name="scratch", bufs=1))

    ps_xfT = ctx.enter_context(tc.tile_pool(name="ps_xfT", bufs=2, space=PS))
    ps_lg1 = ctx.enter_context(tc.tile_pool(name="ps_lg1", bufs=1, space=PS))
    ps_lgT = ctx.enter_context(tc.tile_pool(name="ps_lgT", bufs=1, space=PS))
    ps_agg = ctx.enter_context(tc.tile_pool(name="ps_agg", bufs=1, space=PS))

    ident_f32 = consts.tile([128, 128], F32)
    make_identity(nc, ident_f32)
    ident = consts.tile([128, 128], F32R)
    nc.vector.tensor_copy(ident, ident_f32)
    wt_sb = consts.tile([128, CCN, K], F32R)
    nc.sync.dma_start(out=wt_sb, in_=wt.rearrange("(cc p) k -> p cc k", p=128))
    cent_sb = consts.tile([K, C], F32)
    nc.sync.dma_start(out=cent_sb, in_=cent)

    def one_pass():
        for s in range(NS):
            xf = xf_pool.tile([128, CCN, L], F32R)
            nc.sync.dma_start(out=xf, in_=x[s].rearrange("(cc p) l -> p cc l", p=128))

            # ---- logits = conv_w @ xf : psum (64, 784), contract C in 4 chunks
            lg1 = ps_lg1.tile([K, L], F32)
            for cc in range(CCN):
                for l0, nl in ((0, 512), (512, L - 512)):
                    nc.tensor.matmul(
                        lg1[:, l0:l0 + nl],
                        lhsT=wt_sb[:, cc, :],
                        rhs=xf[:, cc, l0:l0 + nl],
                        start=(cc == 0),
                        stop=(cc == CCN - 1),
                    )
            lg_sb = lg_pool.tile([K, L], F32R)
            nc.scalar.copy(out=lg_sb, in_=lg1)

            # ---- transpose logits chunks: (64, lsz) -> (lsz, 64)
            lgT = ps_lgT.tile([128, NLC, K], F32R)
            for lc in range(NLC):
                l0 = lc * 128
                lsz = min(128, L - l0)
                if lsz < 128:
                    # pad rows so full-tile softmax ops read initialized data
                    # (partition offsets must be 32-aligned, so clear all 128
                    # rows first; the transpose then overwrites the valid ones)
                    nc.vector.memset(lgT[:, lc, :].bitcast(mybir.dt.uint32), 0)
                nc.tensor.transpose(
                    lgT[:lsz, lc, :],
                    lg_sb[:, l0:l0 + lsz],
                    ident[:K, :K],
                )

            # ---- transpose xf chunks: (128c, lsz) -> (lsz, 128c), 4 cc per lc
            xfT = xfT_pool.tile([128, NLC, C + 2], F32R)
            nc.vector.memset(xfT[:, :, C:C + 2].bitcast(mybir.dt.uint32), 0x3F800000)
            for lc in range(NLC):
                l0 = lc * 128
                lsz = min(128, L - l0)
                pxf = ps_xfT.tile([128, C], F32R)
                for cc in range(CCN):
                    nc.tensor.transpose(
                        pxf[:lsz, cc * 128:(cc + 1) * 128],
                        xf[:, cc, l0:l0 + lsz],
                        ident,
                    )
                # PSUM -> SBUF copies, split across DVE and ACT
                eng = nc.vector if (lc % 2 == 0) else nc.scalar
                if eng is nc.vector:
                    nc.vector.tensor_copy(xfT[:lsz, lc, 0:C], pxf[:lsz, :])
                else:
                    nc.scalar.copy(out=xfT[:lsz, lc, 0:C], in_=pxf[:lsz, :])

            # ---- softmax over K (free dim) on lgT
            negmax = small.tile([128, NLC], F32, tag="negmax")
            ssum = small.tile([128, NLC], F32, tag="ssum")
            rs = small.tile([128, NLC], F32, tag="rs")
            aT = aT_pool.tile([128, NLC, K], F32R)
            nc.vector.reduce_max(
                out=negmax, in_=lgT, axis=mybir.AxisListType.X, negate=True
            )
            for lc in range(NLC):
                nc.scalar.activation(
                    out=aT[:, lc, :],
                    in_=lgT[:, lc, :],
                    func=mybir.ActivationFunctionType.Exp,
                    bias=negmax[:, lc:lc + 1],
                    scale=1.0,
                    accum_out=ssum[:, lc:lc + 1],
                )
            nc.vector.reciprocal(out=rs, in_=ssum)
            nc.vector.tensor_mul(aT, aT, rs.to_broadcast((128, NLC, K)))

            # ---- agg = a @ [xfT | 1] : psum (64, 256) + (64, 257)
            agg_a = ps_agg.tile([K, 256], F32, tag="agg_a")
            agg_b = ps_agg.tile([K, 258], F32, tag="agg_b")
            for lc in range(NLC):
                lsz = min(128, L - lc * 128)
                nc.tensor.matmul(
                    agg_a,
                    lhsT=aT[:lsz, lc, :],
                    rhs=xfT[:lsz, lc, 0:256],
                    start=(lc == 0),
                    stop=(lc == NLC - 1),
                )
                nc.tensor.matmul(
                    agg_b,
                    lhsT=aT[:lsz, lc, :],
                    rhs=xfT[:lsz, lc, 256:C + 2],
                    start=(lc == 0),
                    stop=(lc == NLC - 1),
                )

            # ---- epilogue: nvlad = asum*cent - agg (= -vlad)
            nvlad = vlad_pool.tile([K, C], F32)
            asum = agg_b[:, 256:257]
            nc.vector.scalar_tensor_tensor(
                out=nvlad[:, 0:256], in0=cent_sb[:, 0:256], scalar=asum,
                in1=agg_a, op0=mybir.AluOpType.mult, op1=mybir.AluOpType.subtract,
            )
            nc.vector.scalar_tensor_tensor(
                out=nvlad[:, 256:C], in0=cent_sb[:, 256:C], scalar=asum,
                in1=agg_b[:, 0:256], op0=mybir.AluOpType.mult,
                op1=mybir.AluOpType.subtract,
            )
            # ss = sum(vlad^2) per row
            sq_dump = scratch.tile([K, C], F32)
            ss = small.tile([K, 1], F32, tag="ss")
            nc.vector.tensor_mul(sq_dump, nvlad, nvlad)
            nc.vector.reduce_sum(out=ss, in_=sq_dump, axis=mybir.AxisListType.X)
            # rn = rsqrt(64*ss) = rsqrt(ss)/8  (intra-norm + global norm folded)
            t1 = small.tile([K, 1], F32, tag="t1")
            rn = small.tile([K, 1], F32, tag="rn")
            rn_neg = small.tile([K, 1], F32, tag="rn_neg")
            nc.scalar.activation(
                out=t1, in_=ss, func=mybir.ActivationFunctionType.Ln,
                bias=0.0, scale=64.0,
            )
            nc.scalar.activation(
                out=rn, in_=t1, func=mybir.ActivationFunctionType.Exp,
                bias=0.0, scale=-0.5,
            )
            nc.vector.tensor_scalar_mul(rn_neg, rn, -1.0)
            out_sb = out_pool.tile([K, C], F32)
            nc.gpsimd.tensor_scalar_mul(out_sb, nvlad, rn_neg)
            nc.sync.dma_start(
                out=out[s].rearrange("(k c) -> k c", k=K), in_=out_sb
            )

    if n_iters == 1:
        one_pass()
    else:
        with tc.For_i(0, n_iters, 1):
            one_pass()


_PROGRAM_CACHE = {}


def _get_program(n_iters: int = 1):
    if n_iters not in _PROGRAM_CACHE:
        _PROGRAM_CACHE[n_iters] = build_program(n_iters)
    return _PROGRAM_CACHE[n_iters]


def make_in_maps(x, conv_w, centroids):
    xr = np.ascontiguousarray(x.reshape(N, C, L), dtype=np.float32)
    wt = np.ascontiguousarray(conv_w.T, dtype=np.float32)
    ct = np.ascontiguousarray(centroids, dtype=np.float32)
    return [
        {"x": np.ascontiguousarray(xr[i * NS:(i + 1) * NS]), "conv_wt": wt, "cent": ct}
        for i in range(NCORES)
    ]


def kernel(x, conv_w, centroids):
    nc = _get_program(1)
    in_maps = make_in_maps(np.asarray(x), np.asarray(conv_w), np.asarray(centroids))
    res = run_bass_kernel_spmd(nc, in_maps, list(range(NCORES)))
    outs = [res.results[i]["out"].reshape(NS, K * C) for i in range(NCORES)]
    return np.concatenate(outs, axis=0).astype(np.float32)


# revision 9
# speedup vs baseline: 1.6132x; 1.6132x over previous
"""NetVLAD pooling kernel for Trainium2 (Bass/Tile), 8-way data parallel over N.

Reference computation (per sample n):
    xf      = x[n].reshape(C, L)                      # C=512, L=784
    logits  = conv_w @ xf                             # (K=64, L)
    a       = softmax(logits, axis=0)                 # over clusters K
    agg     = a @ xf.T                                # (K, C)
    vlad    = agg - a.sum(-1)[:, None] * centroids    # (K, C)
    vlad   /= ||vlad||_2 rows                         # intra-norm over C
    v       = vlad.flatten() / ||vlad.flatten()||_2   # global norm

Layout strategy per core (8 samples):
  - x[s] loaded naturally as (128p, 4cc, 784l), f32r end-to-end so matmuls
    run at full PE rate (fp32 is 4 cyc/row, f32r is 1).
  - logits via PE with conv_w^T chunks stationary -> psum (64, 784), ACT copy
    to SBUF.
  - logits transposed chunkwise on PE -> logitsT (l-part, K-free); softmax is
    then a free-dim reduction (max/exp/sum) per row.
  - x transposed chunkwise on PE -> xfT (l-part, C-free) with two ones
    columns appended so the agg matmul also yields a.sum() (258-wide rhs
    keeps the moving free dim even, required by codegen).
  - agg = a @ [xfT|1] via PE contracting l in 7 chunks of <=128.
  - epilogue: -vlad = asum*cent - agg (fused scalar_tensor_tensor), row
    sum-of-squares via ACT Square+accum_out (Square is a filler function in
    every ACT table set -> no table reload), and the rsqrt for ALL samples is
    batched at the end as exp(-0.5*ln(64*ss)) so the Ln/Exp table sets load
    once per pass instead of per sample. rsqrt(64*ss) = rsqrt(ss)/8 folds the
    global norm in (the flattened norm of row-normalized vlad is 8 up to fp32
    rounding). Final scale on ACT (copy with per-partition scale).
  - GPSIMD is kept out of the steady-state path entirely (its tensor ops and
    semaphore handling are ~50x slower than DVE's).
"""

import numpy as np
from contextlib import ExitStack

import concourse.bass as bass
import concourse.tile as tile
from concourse import bacc, mybir
from concourse.bass_utils import run_bass_kernel_spmd
from concourse.masks import make_identity

N, C, HH, WW, K = 64, 512, 28, 28, 64
L = HH * WW            # 784
NCORES = 8
NS = N // NCORES       # 8 samples per core
CCN = C // 128         # 4 channel chunks
NLC = (L + 127) // 128  # 7 l-chunks (6x128 + 1x16)
F32 = mybir.dt.float32
F32R = mybir.dt.float32r
U32 = mybir.dt.uint32
PS = bass.MemorySpace.PSUM
AF = mybir.ActivationFunctionType


def build_program(n_iters: int = 1):
    nc = bacc.Bacc("TRN2", target_bir_lowering=False, debug=False)
    x_d = nc.dram_tensor("x", [NS, C, L], F32R, kind="ExternalInput")
    wt_d = nc.dram_tensor("conv_wt", [C, K], F32R, kind="ExternalInput")
    cent_d = nc.dram_tensor("cent", [K, C], F32, kind="ExternalInput")
    out_d = nc.dram_tensor("out", [NS, K * C], F32, kind="ExternalOutput")

    with tile.TileContext(nc) as tc, ExitStack() as ctx:
        _build_body(ctx, tc, x_d.ap(), wt_d.ap(), cent_d.ap(), out_d.ap(), n_iters)
    nc.compile()
    return nc


def _build_body(ctx, tc, x, wt, cent, out, n_iters):
    nc = tc.nc

    consts = ctx.enter_context(tc.tile_pool(name="consts", bufs=1))
    xf_pool = ctx.enter_context(tc.tile_pool(name="xf", bufs=2))
    xfT_pool = ctx.enter_context(tc.tile_pool(name="xfT", bufs=2))
    lg_pool = ctx.enter_context(tc.tile_pool(name="lg", bufs=2))
    aT_pool = ctx.enter_context(tc.tile_pool(name="aT", bufs=2))
    small = ctx.enter_context(tc.tile_pool(name="small", bufs=2))
    # one slot per sample tag; all 8 nvlad tiles stay live until the batched
    # normalization at the end of the pass
    vlad_pool = ctx.enter_context(tc.tile_pool(name="vlad", bufs=1))
    out_pool = ctx.enter_context(tc.tile_pool(name="outp", bufs=3))
    scratch = ctx.enter_context(tc.tile_pool(name="scratch", bufs=2))
    batch = ctx.enter_context(tc.tile_pool(name="batch", bufs=2))

    ps_xfT = ctx.enter_context(tc.tile_pool(name="ps_xfT", bufs=2, space=PS))
    ps_lg1 = ctx.enter_context(tc.tile_pool(name="ps_lg1", bufs=1, space=PS))
    ps_lgT = ctx.enter_context(tc.tile_pool(name="ps_lgT", bufs=2, space=PS))
    ps_agg = ctx.enter_context(tc.tile_pool(name="ps_agg", bufs=1, space=PS))

    ident_f32 = consts.tile([128, 128], F32)
    make_identity(nc, ident_f32)
    ident = consts.tile([128, 128], F32R)
    nc.vector.tensor_copy(ident, ident_f32)
    wt_sb = consts.tile([128, CCN, K], F32R)
    nc.sync.dma_start(out=wt_sb, in_=wt.rearrange("(cc p) k -> p cc k", p=128))
    cent_sb = consts.tile([K, C], F32)
    nc.sync.dma_start(out=cent_sb, in_=cent)

    def one_pass():
        nvlads = []
        ss_all = batch.tile([K, NS], F32, tag="ss_all")
        for s in range(NS):
            xf = xf_pool.tile([128, CCN, L], F32R)
            nc.sync.dma_start(out=xf, in_=x[s].rearrange("(cc p) l -> p cc l", p=128))

            # ---- logits = conv_w @ xf : psum (64, 784), contract C in 4 chunks
            lg1 = ps_lg1.tile([K, L], F32)
            for cc in range(CCN):
                for l0, nl in ((0, 512), (512, L - 512)):
                    nc.tensor.matmul(
                        lg1[:, l0:l0 + nl],
                        lhsT=wt_sb[:, cc, :],
                        rhs=xf[:, cc, l0:l0 + nl],
                        start=(cc == 0),
                        stop=(cc == CCN - 1),
                    )
            lg_sb = lg_pool.tile([K, L], F32R)
            nc.scalar.copy(out=lg_sb, in_=lg1)

            # ---- transpose logits chunks: (64, lsz) -> (lsz, 64)
            lgT = ps_lgT.tile([128, NLC, K], F32R)
            for lc in range(NLC):
                l0 = lc * 128
                lsz = min(128, L - l0)
                if lsz < 128:
                    # pad rows so full-tile softmax ops read initialized data
                    nc.vector.memset(lgT[:, lc, :].bitcast(U32), 0)
                nc.tensor.transpose(
                    lgT[:lsz, lc, :],
                    lg_sb[:, l0:l0 + lsz],
                    ident[:K, :K],
                )

            # ---- transpose xf chunks: (128c, lsz) -> (lsz, 128c), 4 cc per lc
            xfT = xfT_pool.tile([128, NLC, C + 2], F32R)
            nc.vector.memset(xfT[:, :, C:C + 2].bitcast(U32), 0x3F800000)
            for lc in range(NLC):
                l0 = lc * 128
                lsz = min(128, L - l0)
                pxf = ps_xfT.tile([128, C], F32R)
                for cc in range(CCN):
                    nc.tensor.transpose(
                        pxf[:lsz, cc * 128:(cc + 1) * 128],
                        xf[:, cc, l0:l0 + lsz],
                        ident,
                    )
                # PSUM -> SBUF copies, split across DVE and ACT
                if lc % 2 == 0:
                    nc.vector.tensor_copy(xfT[:lsz, lc, 0:C], pxf[:lsz, :])
                else:
                    nc.scalar.copy(out=xfT[:lsz, lc, 0:C], in_=pxf[:lsz, :])

            # ---- softmax over K (free dim) on lgT
            negmax = small.tile([128, NLC], F32, tag="negmax")
            ssum = small.tile([128, NLC], F32, tag="ssum")
            rs = small.tile([128, NLC], F32, tag="rs")
            aT = aT_pool.tile([128, NLC, K], F32R)
            nc.vector.reduce_max(
                out=negmax, in_=lgT, axis=mybir.AxisListType.X, negate=True
            )
            for lc in range(NLC):
                nc.scalar.activation(
                    out=aT[:, lc, :],
                    in_=lgT[:, lc, :],
                    func=AF.Exp,
                    bias=negmax[:, lc:lc + 1],
                    scale=1.0,
                )
            nc.vector.reduce_sum(out=ssum, in_=aT, axis=mybir.AxisListType.X)
            nc.vector.reciprocal(out=rs, in_=ssum)
            for lc in range(NLC):
                nc.vector.tensor_scalar_mul(
                    aT[:, lc, :], aT[:, lc, :], rs[:, lc:lc + 1]
                )

            # ---- agg = a @ [xfT | 1] : psum (64, 256) + (64, 258)
            agg_a = ps_agg.tile([K, 256], F32, tag="agg_a")
            agg_b = ps_agg.tile([K, 258], F32, tag="agg_b")
            for lc in range(NLC):
                lsz = min(128, L - lc * 128)
                nc.tensor.matmul(
                    agg_a,
                    lhsT=aT[:lsz, lc, :],
                    rhs=xfT[:lsz, lc, 0:256],
                    start=(lc == 0),
                    stop=(lc == NLC - 1),
                )
                nc.tensor.matmul(
                    agg_b,
                    lhsT=aT[:lsz, lc, :],
                    rhs=xfT[:lsz, lc, 256:C + 2],
                    start=(lc == 0),
                    stop=(lc == NLC - 1),
                )

            # ---- epilogue part 1: nvlad = asum*cent - agg (= -vlad), ss
            nvlad = vlad_pool.tile([K, C], F32, tag=f"nvlad{s}")
            asum = agg_b[:, 256:257]
            nc.vector.scalar_tensor_tensor(
                out=nvlad[:, 0:256], in0=cent_sb[:, 0:256], scalar=asum,
                in1=agg_a, op0=mybir.AluOpType.mult, op1=mybir.AluOpType.subtract,
            )
            nc.vector.scalar_tensor_tensor(
                out=nvlad[:, 256:C], in0=cent_sb[:, 256:C], scalar=asum,
                in1=agg_b[:, 0:256], op0=mybir.AluOpType.mult,
                op1=mybir.AluOpType.subtract,
            )
            # ss[s] = sum(vlad^2) per row; Square is in every ACT table set
            sq_dump = scratch.tile([K, C], F32, tag="sq")
            nc.scalar.activation(
                out=sq_dump, in_=nvlad, func=AF.Square,
                accum_out=ss_all[:, s:s + 1],
            )
            nvlads.append(nvlad)

        # ---- epilogue part 2 (batched): rn = -rsqrt(64*ss) for all samples
        t1 = batch.tile([K, NS], F32, tag="t1")
        rn = batch.tile([K, NS], F32, tag="rn")
        rn_neg = batch.tile([K, NS], F32, tag="rn_neg")
        nc.scalar.activation(out=t1, in_=ss_all, func=AF.Ln, bias=0.0, scale=64.0)
        nc.scalar.activation(out=rn, in_=t1, func=AF.Exp, bias=0.0, scale=-0.5)
        nc.vector.tensor_scalar_mul(rn_neg, rn, -1.0)
        for s in range(NS):
            out_sb = out_pool.tile([K, C], F32)
            nc.scalar.mul(out_sb, nvlads[s], rn_neg[:, s:s + 1])
            nc.sync.dma_start(
                out=out[s].rearrange("(k c) -> k c", k=K), in_=out_sb
            )

    if n_iters == 1:
        one_pass()
    else:
        with tc.For_i(0, n_iters, 1):
            one_pass()


_PROGRAM_CACHE = {}


def _get_program(n_iters: int = 1):
    if n_iters not in _PROGRAM_CACHE:
        _PROGRAM_CACHE[n_iters] = build_program(n_iters)
    return _PROGRAM_CACHE[n_iters]


def make_in_maps(x, conv_w, centroids):
    xr = np.ascontiguousarray(x.reshape(N, C, L), dtype=np.float32)
    wt = np.ascontiguousarray(conv_w.T, dtype=np.float32)
    ct = np.ascontiguousarray(centroids, dtype=np.float32)
    return [
        {"x": np.ascontiguousarray(xr[i * NS:(i + 1) * NS]), "conv_wt": wt, "cent": ct}
        for i in range(NCORES)
    ]


def kernel(x, conv_w, centroids):
    nc = _get_program(1)
    in_maps = make_in_maps(np.asarray(x), np.asarray(conv_w), np.asarray(centroids))
    res = run_bass_kernel_spmd(nc, in_maps, list(range(NCORES)))
    outs = [res.results[i]["out"].reshape(NS, K * C) for i in range(NCORES)]
    return np.concatenate(outs, axis=0).astype(np.float32)


# revision 10
# speedup vs baseline: 1.6545x; 1.0256x over previous
"""NetVLAD pooling kernel for Trainium2 (Bass/Tile), 8-way data parallel over N.

Reference computation (per sample n):
    xf      = x[n].reshape(C, L)                      # C=512, L=784
    logits  = conv_w @ xf                             # (K=64, L)
    a       = softmax(logits, axis=0)                 # over clusters K
    agg     = a @ xf.T                                # (K, C)
    vlad    = agg - a.sum(-1)[:, None] * centroids    # (K, C)
    vlad   /= ||vlad||_2 rows                         # intra-norm over C
    v       = vlad.flatten() / ||vlad.flatten()||_2   # global norm

Layout strategy per core (8 samples):
  - x[s] loaded naturally as (128p, 4cc, 784l), f32r end-to-end so matmuls
    run at full PE rate (fp32 is 4 cyc/row, f32r is 1).
  - logits via PE with conv_w^T chunks stationary -> psum (64, 784), ACT copy
    to SBUF.
  - logits transposed chunkwise on PE -> logitsT (l-part, K-free); softmax is
    then a free-dim reduction (max/exp/sum) per row.
  - x transposed chunkwise on PE -> xfT (l-part, C-free) with two ones
    columns appended so the agg matmul also yields a.sum() (258-wide rhs
    keeps the moving free dim even, required by codegen).
  - agg = a @ [xfT|1] via PE contracting l in 7 chunks of <=128.
  - epilogue: -vlad = asum*cent - agg (fused scalar_tensor_tensor), row
    sum-of-squares via ACT Square+accum_out (Square is a filler function in
    every ACT table set -> no table reload), and the rsqrt for ALL samples is
    batched at the end as exp(-0.5*ln(64*ss)) so the Ln/Exp table sets load
    once per pass instead of per sample. rsqrt(64*ss) = rsqrt(ss)/8 folds the
    global norm in (the flattened norm of row-normalized vlad is 8 up to fp32
    rounding). Final scale on ACT (copy with per-partition scale).
  - GPSIMD is kept out of the steady-state path entirely (its tensor ops and
    semaphore handling are ~50x slower than DVE's).
"""

import numpy as np
from contextlib import ExitStack

import concourse.bass as bass
import concourse.tile as tile
from concourse import bacc, mybir
from concourse.bass_utils import run_bass_kernel_spmd
from concourse.masks import make_identity

N, C, HH, WW, K = 64, 512, 28, 28, 64
L = HH * WW            # 784
NCORES = 8
NS = N // NCORES       # 8 samples per core
CCN = C // 128         # 4 channel chunks
NLC = (L + 127) // 128  # 7 l-chunks (6x128 + 1x16)
F32 = mybir.dt.float32
F32R = mybir.dt.float32r
U32 = mybir.dt.uint32
PS = bass.MemorySpace.PSUM
AF = mybir.ActivationFunctionType


def build_program(n_iters: int = 1):
    nc = bacc.Bacc("TRN2", target_bir_lowering=False, debug=False)
    x_d = nc.dram_tensor("x", [NS, C, L], F32R, kind="ExternalInput")
    wt_d = nc.dram_tensor("conv_wt", [C, K], F32R, kind="ExternalInput")
    cent_d = nc.dram_tensor("cent", [K, C], F32, kind="ExternalInput")
    out_d = nc.dram_tensor("out", [NS, K * C], F32, kind="ExternalOutput")

    with tile.TileContext(nc) as tc, ExitStack() as ctx:
        _build_body(ctx, tc, x_d.ap(), wt_d.ap(), cent_d.ap(), out_d.ap(), n_iters)
    nc.compile()
    return nc


def _build_body(ctx, tc, x, wt, cent, out, n_iters):
    nc = tc.nc

    consts = ctx.enter_context(tc.tile_pool(name="consts", bufs=1))
    xf_pool = ctx.enter_context(tc.tile_pool(name="xf", bufs=2))
    xfT_pool = ctx.enter_context(tc.tile_pool(name="xfT", bufs=2))
    lg_pool = ctx.enter_context(tc.tile_pool(name="lg", bufs=2))
    aT_pool = ctx.enter_context(tc.tile_pool(name="aT", bufs=2))
    small = ctx.enter_context(tc.tile_pool(name="small", bufs=2))
    # one slot per sample tag; all 8 nvlad tiles stay live until the batched
    # normalization at the end of the pass
    vlad_pool = ctx.enter_context(tc.tile_pool(name="vlad", bufs=1))
    out_pool = ctx.enter_context(tc.tile_pool(name="outp", bufs=2))
    scratch = ctx.enter_context(tc.tile_pool(name="scratch", bufs=2))
    batch = ctx.enter_context(tc.tile_pool(name="batch", bufs=2))

    ps_xfT = ctx.enter_context(tc.tile_pool(name="ps_xfT", bufs=2, space=PS))
    ps_lg1 = ctx.enter_context(tc.tile_pool(name="ps_lg1", bufs=1, space=PS))
    ps_lgT = ctx.enter_context(tc.tile_pool(name="ps_lgT", bufs=2, space=PS))
    ps_agg = ctx.enter_context(tc.tile_pool(name="ps_agg", bufs=1, space=PS))

    ident_f32 = consts.tile([128, 128], F32)
    make_identity(nc, ident_f32)
    ident = consts.tile([128, 128], F32R)
    nc.vector.tensor_copy(ident, ident_f32)
    wt_sb = consts.tile([128, CCN, K], F32R)
    nc.sync.dma_start(out=wt_sb, in_=wt.rearrange("(cc p) k -> p cc k", p=128))
    cent_sb = consts.tile([K, C], F32)
    nc.sync.dma_start(out=cent_sb, in_=cent)

    def one_pass():
        nvlads = []
        ss_all = batch.tile([K, NS], F32, tag="ss_all")
        for s in range(NS):
            xf = xf_pool.tile([128, CCN, L], F32R)
            nc.sync.dma_start(out=xf, in_=x[s].rearrange("(cc p) l -> p cc l", p=128))

            # ---- logits (psum (64,784), contract C in 4 chunks) interleaved
            # with xf transposes: transpose-mode MMs don't register as PE
            # activity for the HAM clock gate, so long transpose-only bursts
            # re-throttle the PE to 1.2 GHz. Breaking them up with regular
            # matmul pieces keeps the clock at 2.4 GHz.
            lg1 = ps_lg1.tile([K, L], F32)
            xfT = xfT_pool.tile([128, NLC, C + 2], F32R)
            nc.vector.memset(xfT[:, :, C:C + 2].bitcast(U32), 0x3F800000)

            def xf_transpose(lc):
                l0 = lc * 128
                lsz = min(128, L - l0)
                pxf = ps_xfT.tile([128, C], F32R)
                for cc in range(CCN):
                    nc.tensor.transpose(
                        pxf[:lsz, cc * 128:(cc + 1) * 128],
                        xf[:, cc, l0:l0 + lsz],
                        ident,
                    )
                # PSUM -> SBUF copies, split across DVE and ACT
                if lc % 2 == 0:
                    nc.vector.tensor_copy(xfT[:lsz, lc, 0:C], pxf[:lsz, :])
                else:
                    nc.scalar.copy(out=xfT[:lsz, lc, 0:C], in_=pxf[:lsz, :])

            def logits_piece(l0, nl):
                for cc in range(CCN):
                    nc.tensor.matmul(
                        lg1[:, l0:l0 + nl],
                        lhsT=wt_sb[:, cc, :],
                        rhs=xf[:, cc, l0:l0 + nl],
                        start=(cc == 0),
                        stop=(cc == CCN - 1),
                    )

            xf_transpose(0)
            xf_transpose(1)
            logits_piece(0, 256)
            xf_transpose(2)
            xf_transpose(3)
            logits_piece(256, 256)
            xf_transpose(4)
            xf_transpose(5)
            logits_piece(512, L - 512)
            xf_transpose(6)

            lg_sb = lg_pool.tile([K, L], F32R)
            nc.scalar.copy(out=lg_sb, in_=lg1)

            # ---- transpose logits chunks: (64, lsz) -> (lsz, 64)
            lgT = ps_lgT.tile([128, NLC, K], F32R)
            for lc in range(NLC):
                l0 = lc * 128
                lsz = min(128, L - l0)
                if lsz < 128:
                    # pad rows so full-tile softmax ops read initialized data
                    nc.vector.memset(lgT[:, lc, :].bitcast(U32), 0)
                nc.tensor.transpose(
                    lgT[:lsz, lc, :],
                    lg_sb[:, l0:l0 + lsz],
                    ident[:K, :K],
                )

            # ---- softmax over K (free dim) on lgT
            negmax = small.tile([128, NLC], F32, tag="negmax")
            ssum = small.tile([128, NLC], F32, tag="ssum")
            rs = small.tile([128, NLC], F32, tag="rs")
            aT = aT_pool.tile([128, NLC, K], F32R)
            nc.vector.reduce_max(
                out=negmax, in_=lgT, axis=mybir.AxisListType.X, negate=True
            )
            for lc in range(NLC):
                nc.scalar.activation(
                    out=aT[:, lc, :],
                    in_=lgT[:, lc, :],
                    func=AF.Exp,
                    bias=negmax[:, lc:lc + 1],
                    scale=1.0,
                )
            nc.vector.reduce_sum(out=ssum, in_=aT, axis=mybir.AxisListType.X)
            nc.vector.reciprocal(out=rs, in_=ssum)
            for lc in range(NLC):
                nc.vector.tensor_scalar_mul(
                    aT[:, lc, :], aT[:, lc, :], rs[:, lc:lc + 1]
                )

            # ---- agg = a @ [xfT | 1] : psum (64, 256) + (64, 258)
            agg_a = ps_agg.tile([K, 256], F32, tag="agg_a")
            agg_b = ps_agg.tile([K, 258], F32, tag="agg_b")
            for lc in range(NLC):
                lsz = min(128, L - lc * 128)
                nc.tensor.matmul(
                    agg_a,
                    lhsT=aT[:lsz, lc, :],
                    rhs=xfT[:lsz, lc, 0:256],
                    start=(lc == 0),
                    stop=(lc == NLC - 1),
                )
                nc.tensor.matmul(
                    agg_b,
                    lhsT=aT[:lsz, lc, :],
                    rhs=xfT[:lsz, lc, 256:C + 2],
                    start=(lc == 0),
                    stop=(lc == NLC - 1),
                )

            # ---- epilogue part 1: nvlad = asum*cent - agg (= -vlad), ss
            nvlad = vlad_pool.tile([K, C], F32, tag=f"nvlad{s}")
            asum = agg_b[:, 256:257]
            nc.vector.scalar_tensor_tensor(
                out=nvlad[:, 0:256], in0=cent_sb[:, 0:256], scalar=asum,
                in1=agg_a, op0=mybir.AluOpType.mult, op1=mybir.AluOpType.subtract,
            )
            nc.vector.scalar_tensor_tensor(
                out=nvlad[:, 256:C], in0=cent_sb[:, 256:C], scalar=asum,
                in1=agg_b[:, 0:256], op0=mybir.AluOpType.mult,
                op1=mybir.AluOpType.subtract,
            )
            # ss[s] = sum(vlad^2) per row; Square is in every ACT table set
            sq_dump = scratch.tile([K, C], F32, tag="sq")
            nc.scalar.activation(
                out=sq_dump, in_=nvlad, func=AF.Square,
                accum_out=ss_all[:, s:s + 1],
            )
            nvlads.append(nvlad)

        # ---- epilogue part 2 (batched): rn = -rsqrt(64*ss) for all samples
        t1 = batch.tile([K, NS], F32, tag="t1")
        rn = batch.tile([K, NS], F32, tag="rn")
        rn_neg = batch.tile([K, NS], F32, tag="rn_neg")
        nc.scalar.activation(out=t1, in_=ss_all, func=AF.Ln, bias=0.0, scale=64.0)
        nc.scalar.activation(out=rn, in_=t1, func=AF.Exp, bias=0.0, scale=-0.5)
        nc.vector.tensor_scalar_mul(rn_neg, rn, -1.0)
        out_all = out_pool.tile([K, NS, C], F32)
        for s in range(NS):
            nc.scalar.mul(out_all[:, s, :], nvlads[s], rn_neg[:, s:s + 1])
        # one 1 MB store: out[s, k*C + c] <- out_all[k, s, c]
        nc.sync.dma_start(
            out=out.rearrange("s (k c) -> k s c", k=K), in_=out_all
        )

    if n_iters == 1:
        one_pass()
    else:
        with tc.For_i(0, n_iters, 1):
            one_pass()


_PROGRAM_CACHE = {}


def _get_program(n_iters: int = 1):
    if n_iters not in _PROGRAM_CACHE:
        _PROGRAM_CACHE[n_iters] = build_program(n_iters)
    return _PROGRAM_CACHE[n_iters]


def make_in_maps(x, conv_w, centroids):
    xr = np.ascontiguousarray(x.reshape(N, C, L), dtype=np.float32)
    wt = np.ascontiguousarray(conv_w.T, dtype=np.float32)
    ct = np.ascontiguousarray(centroids, dtype=np.float32)
    return [
        {"x": np.ascontiguousarray(xr[i * NS:(i + 1) * NS]), "conv_wt": wt, "cent": ct}
        for i in range(NCORES)
    ]


def kernel(x, conv_w, centroids):
    nc = _get_program(1)
    in_maps = make_in_maps(np.asarray(x), np.asarray(conv_w), np.asarray(centroids))
    res = run_bass_kernel_spmd(nc, in_maps, list(range(NCORES)))
    outs = [res.results[i]["out"].reshape(NS, K * C) for i in range(NCORES)]
    return np.concatenate(outs, axis=0).astype(np.float32)
